# revision 20
# baseline (speedup 1.0000x reference)
"""Causal self-attention (B=4, T=2048, C=768, 12 heads) on 8 trn2 NeuronCores.

Sharding: core c handles batch b = c//2 and head-group hg = c%2 (6 heads each).
Each core computes its 6 heads end-to-end plus its slice of the output
projection; the two head-group partial projections per batch are summed on the
host (one 6 MB add per batch).

Per-core pipeline (matmuls in bf16 with fp32 PSUM accumulation — matches the
bf16-native numerics envelope for dense transformers; softmax in fp32):
  - xT chunks DMA'd per 128-row contraction slice (host-prepared bf16)
  - qT/kT = W_qk^T x^T via PE (heads pair-packed into 128-partition tiles)
  - V in natural [t, hd] layout via PE with xT as the stationary operand,
    with a ones column appended per head for the softmax denominator
  - S^T[k,q] = K Q^T per head, two heads per PE slot via tile_position row
    packing (contraction dim = hd = 64); S and PV matmuls batched in groups
    of two k-tiles to cut PE array mode switches
  - exp on ACT straight out of PSUM (two heads per op) -> bf16 E; causal
    zero-fill via affine_select on GpSimd (diagonal tiles only)
  - PV accumulation in PSUM over k tiles (M=65: 64 value dims + denominator)
  - normalization: DVE reciprocal straight from PSUM, GpSimd
    partition_broadcast (no DRAM bounce), one in-place DVE multiply; the
    head-B partition shift 0:64 -> 64:128 uses a DVE stream_shuffle
  - output projection per q-chunk from the transposed yT layout, DMA out
"""

import ml_dtypes
import numpy as np

import concourse.bacc as bacc
import concourse.mybir as mybir
import concourse.tile as tile
from concourse.bass_utils import run_bass_kernel_spmd

F32 = mybir.dt.float32
BF16 = mybir.dt.bfloat16
AF = mybir.ActivationFunctionType

B, T, C = 4, 2048, 768
NH, HD = 12, 64
TC = 4  # t-chunks of 512
CC = 6  # contraction chunks of 128 over C
N_TT = 16  # t tiles of 128

IDENT32 = list(range(32))

_SEL = np.zeros((33, 128), dtype=ml_dtypes.bfloat16)
_SEL[0, 0:64] = 1
_SEL[32, 64:128] = 1

_nc_cache = {}


def _build(with_bias: bool):
    nc = bacc.Bacc(None, target_bir_lowering=False)
    xt_d = nc.dram_tensor("xt", [C, T], BF16, kind="ExternalInput")
    wqk = nc.dram_tensor("wqk", [C, 768], BF16, kind="ExternalInput")
    wv = nc.dram_tensor("wv", [C, 384], BF16, kind="ExternalInput")
    wp = nc.dram_tensor("wp", [384, C], BF16, kind="ExternalInput")
    sel_d = nc.dram_tensor("sel", [33, 128], BF16, kind="ExternalInput")
    if with_bias:
        bqk = nc.dram_tensor("bqk", [1, 768], BF16, kind="ExternalInput")
        bv = nc.dram_tensor("bv", [1, 384], BF16, kind="ExternalInput")
    out = nc.dram_tensor("out", [T, C], F32, kind="ExternalOutput")

    with tile.TileContext(nc) as tc, nc.allow_low_precision(
        reason="bf16 matmul operands are intentional"
    ):
        with (
            tc.tile_pool(name="const", bufs=1) as const,
            tc.tile_pool(name="xt", bufs=2) as xtp,
            tc.tile_pool(name="big", bufs=1) as big,
            tc.tile_pool(name="E", bufs=6) as epool,
            tc.tile_pool(name="norm", bufs=2) as npool,
            tc.tile_pool(name="ost", bufs=3) as opool,
            tc.tile_pool(name="ps", bufs=1, space="PSUM") as ps,
        ):
            # PE warmup first: dummy matmuls with no input deps keep the
            # p-state high while the first DMAs land (wdum memset leads the
            # DVE queue so LDWEIGHTS isn't stuck behind other memsets)
            wdum = const.tile([128, 256], BF16, name="wdum")
            nc.vector.memset(wdum.bitcast(F32)[:, 0:128], 0.0)
            wps = ps.tile([128, 512], F32, tag="mm", bufs=2, name="warmps")
            for _ in range(12):
                nc.tensor.matmul(wps[:, 0:256], wdum[:, 0:128], wdum, start=True, stop=True)

            # ---------------- constants ----------------
            onecol_f = const.tile([128, 8], F32)
            nc.vector.memset(onecol_f, 1.0)
            onecol_bf = const.tile([128, 8], BF16)
            nc.vector.tensor_copy(onecol_bf, onecol_f)

            # warm the ACT exp table while DMAs run
            warm_f = const.tile([1, 8], F32)
            nc.scalar.activation(warm_f, onecol_f[0:1, :], AF.Exp, scale=1.0)

            # selector for the PE-side recip broadcast: out row j of the
            # bcast matmul takes recip row 0 (head A) for j<64, row 1 for
            # j>=64 (host-prepared; memset can't write at partition base 1)
            sel = const.tile([33, 128], BF16, name="sel")
            nc.sync.dma_start(out=sel, in_=sel_d[:, :])

            # denominator staging: head A at partition 0, head B at partition
            # 32 (SBUF writes need 32-aligned bases); junk rows kept finite
            # (memset 1.0) so the recip/bcast matmul stays NaN-free
            den_tiles = [
                const.tile([33, 512], F32, name=f"denp{i}") for i in range(4)
            ]
            for dt_ in den_tiles:
                nc.vector.memset(dt_, 1.0)

            if with_bias:
                ones_f = const.tile([1, 512], F32)
                nc.vector.memset(ones_f, 1.0)
                ones_bf = const.tile([1, 512], BF16)
                nc.vector.tensor_copy(ones_bf, ones_f)

            warm_sb = const.tile([1, 8], F32, name="warmsb")
            nc.vector.tensor_copy(warm_sb, wps[0:1, 0:8])

            # ---------------- weights (bf16, host-prepared) ----------------
            # per-chunk DMAs so the first V/QK matmuls start as soon as their
            # slices land; wv + the first x chunk are needed first, wp last
            wv_t = const.tile([128, CC, 384], BF16, name="wv_t")
            wv_r = wv.rearrange("(cc p) n -> p cc n", p=128)
            for cc in range(CC):
                nc.sync.dma_start(out=wv_t[:, cc, :], in_=wv_r[:, cc, :])
            wv_bf = [wv_t[:, cc, :] for cc in range(CC)]

            xt_r = xt_d.rearrange("(cc p) t -> p cc t", p=128)

            def dma_xt(qc, splits=1):
                xt_t = xtp.tile([128, CC, 512], BF16, tag="xt", name=f"xt_{qc}")
                w = 512 // splits
                for sp in range(splits):
                    for cc in range(CC):
                        nc.sync.dma_start(
                            out=xt_t[:, cc, sp * w : (sp + 1) * w],
                            in_=xt_r[:, cc, qc * 512 + sp * w : qc * 512 + (sp + 1) * w],
                        )
                return xt_t

            xt0 = dma_xt(0)

            wqk_t = const.tile([128, CC, 768], BF16, name="wqk_t")
            wqk_r = wqk.rearrange("(cc p) n -> p cc n", p=128)
            for cc in range(CC):
                nc.sync.dma_start(out=wqk_t[:, cc, :], in_=wqk_r[:, cc, :])
            wqk_bf = [wqk_t[:, cc, :] for cc in range(CC)]
            wp_t = const.tile([128, 3, 768], BF16, name="wp_t")
            wp_r = wp.rearrange("(hp p) n -> p hp n", p=128)
            for hp in range(3):
                nc.sync.dma_start(out=wp_t[:, hp, :], in_=wp_r[:, hp, :])
            wp_bf = [wp_t[:, hp, :] for hp in range(3)]
            if with_bias:
                bqk_bf = const.tile([1, 768], BF16)
                nc.sync.dma_start(out=bqk_bf, in_=bqk[:, :])
                bv_bf = const.tile([1, 384], BF16)
                nc.sync.dma_start(out=bv_bf, in_=bv[:, :])

            # persistent big tiles
            qkT = [big.tile([128, T], BF16, name=f"qkT{ct}") for ct in range(6)]
            v_sb = [big.tile([128, 390], BF16, name=f"v{tt}") for tt in range(N_TT)]
            yT3 = [big.tile([128, T], BF16, name=f"yT{hp}") for hp in range(3)]

            def emit_a_phase(qc, xt_t):
                """V and qT/kT matmuls for chunk qc."""
                xt_tiles = [xt_t[:, cc, :] for cc in range(CC)]
                for tt4 in range(4):
                    tt = qc * 4 + tt4
                    v_ps = ps.tile([128, 384], F32, tag="mm", bufs=2, name=f"vps{tt}")
                    for cc in range(CC):
                        nc.tensor.matmul(
                            v_ps,
                            xt_tiles[cc][:, tt4 * 128 : (tt4 + 1) * 128],
                            wv_bf[cc],
                            start=(cc == 0),
                            stop=(cc == CC - 1 and not with_bias),
                        )
                    if with_bias:
                        nc.tensor.matmul(
                            v_ps, ones_bf[:, 0:128], bv_bf, start=False, stop=True
                        )
                    vv = v_sb[tt].rearrange("p (h w) -> p h w", w=65)
                    nc.vector.tensor_copy(
                        vv[:, :, 0:64], v_ps.rearrange("p (h w) -> p h w", w=64)
                    )
                    nc.vector.tensor_copy(vv[:, :, 64], onecol_bf[:, 0:6])
                for ct in range(6):
                    qk_ps = ps.tile(
                        [128, 512], F32, tag="mm", bufs=2, name=f"qkps{qc}_{ct}"
                    )
                    for cc in range(CC):
                        nc.tensor.matmul(
                            qk_ps,
                            wqk_bf[cc][:, ct * 128 : (ct + 1) * 128],
                            xt_tiles[cc],
                            start=(cc == 0),
                            stop=(cc == CC - 1 and not with_bias),
                        )
                    if with_bias:
                        nc.tensor.matmul(
                            qk_ps,
                            bqk_bf[:, ct * 128 : (ct + 1) * 128],
                            ones_bf,
                            start=False,
                            stop=True,
                        )
                    nc.vector.tensor_copy(qkT[ct][:, qc * 512 : (qc + 1) * 512], qk_ps)

            def emit_attention(qc, deferred_norms, mid_cb=None):
                """Attention for chunk qc. deferred_norms: closures from the
                previous chunk, sprinkled between hp streams so their
                DVE/PE-broadcast work overlaps our PE work. For the last
                chunk they are drained early and mid_cb (the previous
                chunk's projection) is emitted after hp1 so only this
                chunk's own projection remains in the tail."""
                q_sl = slice(qc * 512, (qc + 1) * 512)
                n_kt = 4 * qc + 4
                last = qc == TC - 1
                out_norms = []
                for hp in range(3):
                    yT_a = ps.tile([65, 512], F32, tag="yT", bufs=2, name=f"ya{qc}_{hp}")
                    yT_b = ps.tile([65, 512], F32, tag="yT", bufs=2, name=f"yb{qc}_{hp}")
                    prev_grp = []  # [(kt, E, f0)] awaiting PV
                    for g in range(n_kt // 2):
                        grp = []
                        for kt in (2 * g, 2 * g + 1):
                            k_sl = slice(kt * 128, (kt + 1) * 128)
                            m = kt - 4 * qc
                            w = 512 - 128 * max(m, 0)  # live column range
                            f0 = 512 - w
                            psS = ps.tile(
                                [128, 1024], F32, tag="S", bufs=2,
                                name=f"s{qc}_{hp}_{kt}",
                            )
                            nc.tensor.matmul(
                                psS[:, f0:512],
                                qkT[3 + hp][0:64, k_sl],
                                qkT[hp][0:64, qc * 512 + f0 : (qc + 1) * 512],
                                start=True,
                                stop=True,
                                tile_position=(0, 0),
                            )
                            nc.tensor.matmul(
                                psS[:, 512 + f0 : 1024],
                                qkT[3 + hp][64:128, k_sl],
                                qkT[hp][64:128, qc * 512 + f0 : (qc + 1) * 512],
                                start=True,
                                stop=True,
                                tile_position=(64, 0),
                            )
                            grp.append((kt, psS, f0, m >= 0))
                        cur_grp = []
                        for kt, psS, f0, diag in grp:
                            E = epool.tile(
                                [128, 1024], BF16, tag="E", name=f"e{qc}_{hp}_{kt}"
                            )
                            psv = psS.rearrange("p (h w) -> p h w", w=512)
                            ev = E.rearrange("p (h w) -> p h w", w=512)
                            nc.scalar.activation(
                                ev[:, :, f0:512], psv[:, :, f0:512], AF.Exp,
                                scale=0.125,
                            )
                            if diag:
                                # keep where q - k = f' - p >= 0; only the
                                # first 128 live columns can be masked
                                nc.gpsimd.affine_select(
                                    out=ev[:, :, f0 : f0 + 128],
                                    in_=ev[:, :, f0 : f0 + 128],
                                    compare_op=mybir.AluOpType.is_ge,
                                    fill=0.0,
                                    base=0,
                                    pattern=[[0, 2], [1, 128]],
                                    channel_multiplier=-1,
                                )
                            cur_grp.append((kt, E, f0))
                        for kt, E, f0 in prev_grp:
                            _pv(nc, v_sb, yT_a, yT_b, hp, kt, E, f0, n_kt)
                        prev_grp = cur_grp
                    for kt, E, f0 in prev_grp:
                        _pv(nc, v_sb, yT_a, yT_b, hp, kt, E, f0, n_kt)

                    # immediate per-hp epilogue: pull denominators down to
                    # partitions 0/1 and values into yT3 (cross-partition-base
                    # PSUM reads are legal on DVE), releasing PSUM fast
                    den = den_tiles[(qc * 3 + hp) % 4]
                    nc.vector.tensor_copy(den[0:1, :], yT_a[64:65, :])
                    nc.vector.tensor_copy(den[32:33, :], yT_b[64:65, :])
                    nc.vector.tensor_copy(yT3[hp][0:64, q_sl], yT_a[0:64, :])
                    nc.vector.tensor_copy(yT3[hp][64:128, q_sl], yT_b[0:64, :])

                    def norm(hp=hp, den=den, qc=qc):
                        rec_f = npool.tile(
                            [33, 512], F32, tag="recf", bufs=2, name=f"rf{qc}_{hp}"
                        )
                        nc.vector.reciprocal_approx_fast(rec_f, den)
                        rec = npool.tile(
                            [33, 512], BF16, tag="rec", bufs=2, name=f"rc{qc}_{hp}"
                        )
                        nc.vector.tensor_copy(rec, rec_f)
                        # broadcast along partitions on the PE: row j of bcps
                        # = rec[0] for j<64, rec[1] for j>=64
                        bcps = ps.tile(
                            [128, 512], F32, tag="mm", bufs=2, name=f"bc{qc}_{hp}"
                        )
                        nc.tensor.matmul(bcps, sel, rec, start=True, stop=True)
                        nc.vector.tensor_mul(
                            yT3[hp][:, q_sl], yT3[hp][:, q_sl], bcps
                        )

                    if last:
                        norm()
                        # drain the previous chunk's norms early: two after
                        # hp0, the last after hp1, then its projection
                        if hp == 0:
                            while len(deferred_norms) > 1:
                                deferred_norms.pop(0)()
                        elif hp == 1:
                            while deferred_norms:
                                deferred_norms.pop(0)()
                            if mid_cb is not None:
                                mid_cb()
                    else:
                        out_norms.append(norm)
                        if deferred_norms:
                            deferred_norms.pop(0)()
                return out_norms

            def emit_proj(qc):
                for tt in range(qc * 4, qc * 4 + 4):
                    t_sl = slice(tt * 128, (tt + 1) * 128)
                    ostage = opool.tile([128, 768], F32, tag="ost")
                    for half in range(2):
                        pp = ps.tile(
                            [128, 384], F32, tag="mm", bufs=2, name=f"pj{tt}_{half}"
                        )
                        for hp in range(3):
                            nc.tensor.matmul(
                                pp,
                                yT3[hp][:, t_sl],
                                wp_bf[hp][:, half * 384 : (half + 1) * 384],
                                start=(hp == 0),
                                stop=(hp == 2),
                            )
                        nc.vector.tensor_copy(
                            ostage[:, half * 384 : (half + 1) * 384], pp
                        )
                    nc.sync.dma_start(out=out[t_sl, :], in_=ostage)

            # ---------------- main loop: qc-major, proj one chunk behind ----
            emit_a_phase(0, xt0)
            norms = []
            for qc in range(TC):
                mid = (lambda: emit_proj(TC - 2)) if qc == TC - 1 else None
                norms = emit_attention(qc, norms, mid_cb=mid)
                if qc < TC - 1:
                    xt_n = dma_xt(qc + 1)
                    emit_a_phase(qc + 1, xt_n)
                if 1 <= qc < TC - 1:
                    emit_proj(qc - 1)
            emit_proj(TC - 1)

    nc.finalize()
    return nc


def _pv(nc, v_sb, yT_a, yT_b, hp, kt, E, f0, n_kt):
    a = 2 * hp
    nc.tensor.matmul(
        yT_a[:, f0:512],
        v_sb[kt][:, a * 65 : (a + 1) * 65],
        E[:, f0:512],
        start=(kt == 0),
        stop=(kt == n_kt - 1),
    )
    nc.tensor.matmul(
        yT_b[:, f0:512],
        v_sb[kt][:, (a + 1) * 65 : (a + 2) * 65],
        E[:, 512 + f0 : 1024],
        start=(kt == 0),
        stop=(kt == n_kt - 1),
    )


def _get_nc(with_bias: bool):
    if with_bias not in _nc_cache:
        _nc_cache[with_bias] = _build(with_bias)
    return _nc_cache[with_bias]


def kernel(x, W_attn, b_attn, W_proj, b_proj, _run_kwargs=None):
    x = np.ascontiguousarray(np.asarray(x, dtype=np.float32))
    W_attn = np.ascontiguousarray(np.asarray(W_attn, dtype=np.float32))
    b_attn = np.ascontiguousarray(np.asarray(b_attn, dtype=np.float32))
    W_proj = np.ascontiguousarray(np.asarray(W_proj, dtype=np.float32))
    b_proj = np.ascontiguousarray(np.asarray(b_proj, dtype=np.float32))

    with_bias = bool(np.any(b_attn))
    nc = _get_nc(with_bias)

    bf = ml_dtypes.bfloat16
    xt_by_b = [np.ascontiguousarray(x[b].T.astype(bf)) for b in range(B)]
    in_maps = []
    for c in range(8):
        b = c // 2
        hg = c % 2
        cs = slice(hg * 384, (hg + 1) * 384)
        wq = W_attn[:, 0:768][:, cs]
        wk = W_attn[:, 768:1536][:, cs]
        wvs = W_attn[:, 1536:2304][:, cs]
        m = {
            "xt": xt_by_b[b],
            "wqk": np.ascontiguousarray(
                np.concatenate([wq, wk], axis=1).astype(bf)
            ),
            "wv": np.ascontiguousarray(wvs.astype(bf)),
            "wp": np.ascontiguousarray(W_proj[cs, :].astype(bf)),
            "sel": _SEL,
        }
        if with_bias:
            m["bqk"] = np.ascontiguousarray(
                np.concatenate([b_attn[0:768][cs], b_attn[768:1536][cs]]).astype(bf)
            )[None, :]
            m["bv"] = np.ascontiguousarray(b_attn[1536:2304][cs].astype(bf))[None, :]
        in_maps.append(m)

    kwargs = _run_kwargs or {}
    res = run_bass_kernel_spmd(nc, in_maps, core_ids=list(range(8)), **kwargs)

    y = np.empty((B, T, C), dtype=np.float32)
    for b in range(B):
        y[b] = res.results[2 * b]["out"] + res.results[2 * b + 1]["out"]
    y += b_proj[None, None, :]
    if kwargs:
        kernel.last_result = res
    return y
